# revision 48
# baseline (speedup 1.0000x reference)
"""ExpressionAttentionLayer Trainium2 kernel (v5).

Math (per reference, algebraically folded):
  fused/q/k projections folded on the host into one [1024,128] weight per
  core; A_bar = softmax(qk)*M / L1 == exp(qk)*M / sum_k(exp(qk)*M) (the
  softmax denominator cancels; logits are tiny so no max-subtraction);
  the key-sum denominator rides as a ones-column appended to V.

Device decomposition: core d = batch d//4, head pair (2*(d%4), 2*(d%4)+1).

v5 vs the 124us baseline (validated piecewise against HW traces):
  * x / Wq / Wk stored fp8e4m3 (halves the 4.2MB x stream; PE speed is
    dtype-independent so this is DMA-only). Weights rescaled by powers of
    two (x64/x16) to stay in fp8 normal range; undone exactly on the
    PSUM->SBUF copy (acc * 2^-6|-4 + bias). DoubleRow is NOT used: HW
    measured 1 row/cycle regardless, and it disables weight-load overlap.
  * Masks preloaded into a static [128, 64, 512] SBUF region with batched
    DMAs (8 tiles per DIRECT2D vs 64 singles at 625ns SP config each).
  * Prologue groups' bias-adds run on ACT (idle until the first exp);
    groups 2/3 are emitted split (q/k on consecutive tiles) to smooth PE.
  * Output DMAs go out on the gpsimd SWDGE queue (sync queue carries only
    the input stream).
  * Warm-up trimmed 13->6 matmuls (HAM needs ~3.4us of activity; the qk
    chains provide the rest) so the first qk chain isn't queued behind it.
  * Tail: reciprocals read the denominator rows straight from PSUM (drops
    two [1,512] copies), and 1/den is transposed via a tiny DRAM
    broadcast bounce instead of 8 serial rank-1 PE matmuls (~3.2us).
"""

import os
import sys
from collections import defaultdict

for _p in ("/opt/trn_rl_repo", "/root/.axon_site/_ro/trn_rl_repo"):
    if os.path.isdir(_p) and _p not in sys.path:
        sys.path.insert(0, _p)

import numpy as np

import concourse.bass as bass
import concourse.mybir as mybir
import concourse.tile as tile
from concourse import bacc
from concourse.bass_utils import run_bass_kernel_spmd

B, S, D, H, HD = 2, 2048, 512, 8, 64
KX = 2 * D
NCH = KX // 128
NG = 4
N_CORES = 8
QB = 512
NQB = S // QB
KT = 128
NKT = S // KT
NT = NQB * NKT
LAG = 4
SCALE = 1.0 / np.sqrt(HD)
QSH, KSH = 64.0, 16.0
NWARM = int(os.environ.get("KERNEL_NWARM", "3"))

f32 = mybir.dt.float32
bf16 = mybir.dt.bfloat16
f8 = mybir.dt.float8e4

M_DT = bf16
P_DT = bf16
X_DT = f8

GP_MOD = int(os.environ.get("KERNEL_GP_MOD", "0"))
GP_OFF = int(os.environ.get("KERNEL_GP_OFF", "3"))
GP_SET = set(T for T in range(NT) if GP_MOD and T % GP_MOD == GP_OFF)

_compiled = None
_last_results = None


def _build():
    nc = bacc.Bacc("TRN2", target_bir_lowering=False, debug=False,
                   num_devices=N_CORES)
    AF = mybir.ActivationFunctionType

    xt = nc.dram_tensor("xt", [128, NG * NCH, QB], X_DT,
                        kind="ExternalInput").ap()
    mt = nc.dram_tensor("mt", [128, NT * QB], M_DT, kind="ExternalInput").ap()
    wq = nc.dram_tensor("wq", [128, NCH, 128], X_DT, kind="ExternalInput").ap()
    wk = nc.dram_tensor("wk", [128, NCH, 128], X_DT, kind="ExternalInput").ap()
    bq = nc.dram_tensor("bq", [128, 1], f32, kind="ExternalInput").ap()
    bk = nc.dram_tensor("bk", [128, 1], f32, kind="ExternalInput").ap()
    v0 = nc.dram_tensor("v0", [128, NKT, HD + 1], P_DT, kind="ExternalInput").ap()
    v1 = nc.dram_tensor("v1", [128, NKT, HD + 1], P_DT, kind="ExternalInput").ap()
    wo0 = nc.dram_tensor("wo0", [HD, D], bf16, kind="ExternalInput").ap()
    wo1 = nc.dram_tensor("wo1", [HD, D], bf16, kind="ExternalInput").ap()
    out = nc.dram_tensor("out", [S, D], f32, kind="ExternalOutput").ap()

    with tile.TileContext(nc) as tc:
        with tc.tile_pool(name="const", bufs=1) as const, \
             tc.tile_pool(name="ep", bufs=8) as ep, \
             tc.tile_pool(name="pp", bufs=8) as pp, \
             tc.tile_pool(name="nsb", bufs=2) as nsb, \
             tc.tile_pool(name="small", bufs=2) as small, \
             tc.tile_pool(name="bcp", bufs=2) as bcp, \
             tc.tile_pool(name="shp", bufs=2) as shp, \
             tc.tile_pool(name="outp", bufs=4) as outp, \
             tc.tile_pool(name="pst", bufs=2, space="PSUM") as pst, \
             tc.tile_pool(name="pacc", bufs=2, space="PSUM") as pacc, \
             tc.tile_pool(name="drp", bufs=2, space="DRAM") as drp:

            # ---- PE warm-up burst + Exp table preload -----------------
            warm_in = const.tile([128, QB], bf16)
            nc.vector.memset(warm_in, 1.0)
            warm_o = const.tile([1, 8], f32)
            for i in range(NWARM):
                warm_ps = pacc.tile([128, QB], f32, tag="qk",
                                    name=f"warm{i}")
                nc.tensor.matmul(warm_ps, warm_in[:, 0:128], warm_in,
                                 start=True, stop=True)
            nc.scalar.activation(warm_o, warm_in[0:1, 0:8], AF.Exp)

            # ---- static input regions ---------------------------------
            xt_s = const.tile([128, NG * NCH, QB], X_DT)
            msk_s = const.tile([128, NT, QB], M_DT)
            wq_s = const.tile([128, NCH, 128], X_DT)
            wk_s = const.tile([128, NCH, 128], X_DT)
            bq_s = const.tile([128, 1], f32)
            bk_s = const.tile([128, 1], f32)
            v0_s = const.tile([128, NKT, HD + 1], P_DT)
            v1_s = const.tile([128, NKT, HD + 1], P_DT)
            wo0_s = const.tile([HD, D], bf16)
            wo1_s = const.tile([HD, D], bf16)

            def issue_xt(g):
                nc.sync.dma_start(out=xt_s[:, g * NCH:(g + 1) * NCH, :],
                                  in_=xt[:, g * NCH:(g + 1) * NCH, :])

            def issue_mask_batch(start, count):
                nc.sync.dma_start(
                    out=msk_s[:, start:start + count, :],
                    in_=mt[:, start * QB:(start + count) * QB])

            # SWDGE warm-up: pay the software queue's startup latency now.
            swdge_warm = drp.tile([1, 8], f32, tag="swarm", name="swarm")
            nc.gpsimd.dma_start(out=swdge_warm, in_=warm_o)

            # sync-queue order tuned so early-needed data isn't stuck
            # behind bulk transfers (the HWDGE queue is FIFO). Weights
            # first (small), then x chunks 0-1 so the first projection
            # chunks can start ~2.5us before the full x group lands.
            nc.sync.dma_start(out=wq_s, in_=wq)
            nc.sync.dma_start(out=wk_s, in_=wk)
            nc.sync.dma_start(out=xt_s[:, 0:2, :], in_=xt[:, 0:2, :])
            nc.sync.dma_start(out=xt_s[:, 2:NCH, :], in_=xt[:, 2:NCH, :])
            nc.sync.dma_start(out=bq_s, in_=bq)
            nc.sync.dma_start(out=bk_s, in_=bk)
            issue_mask_batch(0, 8)
            issue_xt(1)
            nc.sync.dma_start(out=v0_s, in_=v0)
            nc.sync.dma_start(out=v1_s, in_=v1)
            issue_xt(2)
            issue_mask_batch(8, 4)
            issue_xt(3)
            issue_mask_batch(12, 4)
            nc.sync.dma_start(out=wo0_s, in_=wo0)
            nc.sync.dma_start(out=wo1_s, in_=wo1)

            qT_g = [const.tile([128, QB], bf16, name=f"qT{g}")
                    for g in range(NG)]
            kT_g = [const.tile([128, QB], bf16, name=f"kT{g}")
                    for g in range(NG)]

            group_accs = {}

            def emit_group_chunks(g, t, lo, hi, on_act=False):
                """Chunks [lo,hi) of one fp8 projection chain (t=0: q,
                t=1: k), emitted piecewise so the Tile scheduler can't
                hoist a whole 1.7us chain ahead of a ready ST and starve
                ACT. The power-of-2 weight rescale is undone on the
                PSUM->SBUF copy (ACT for prologue pieces, DVE in-loop)."""
                w_s, b_s, sh, dst = (
                    (wq_s, bq_s, 1.0 / QSH, qT_g[g]) if t == 0
                    else (wk_s, bk_s, 1.0 / KSH, kT_g[g]))
                if lo == 0:
                    group_accs[(g, t)] = pacc.tile([128, QB], f32, tag="qk",
                                                   name=f"qk{t}_{g}")
                acc = group_accs[(g, t)]
                for i in range(lo, hi):
                    nc.tensor.matmul(
                        acc, w_s[:, i, :], xt_s[:, g * NCH + i, :],
                        start=(i == 0), stop=(i == NCH - 1))
                if hi == NCH:
                    if on_act:
                        if t == 1:
                            # ST(0) only needs kT cols 0-127; unblock it
                            # before the rest of the bias copy
                            nc.scalar.activation(dst[:, 0:KT],
                                                 acc[:, 0:KT],
                                                 AF.Identity,
                                                 bias=b_s, scale=sh)
                            nc.scalar.activation(dst[:, KT:QB],
                                                 acc[:, KT:QB],
                                                 AF.Identity,
                                                 bias=b_s, scale=sh)
                        else:
                            nc.scalar.activation(dst, acc, AF.Identity,
                                                 bias=b_s, scale=sh)
                    else:
                        nc.vector.tensor_scalar(dst, acc, sh, b_s,
                                                mybir.AluOpType.mult,
                                                mybir.AluOpType.add)

            def emit_group(g, on_act=False):
                for t in (0, 1):
                    emit_group_chunks(g, t, 0, 4, on_act)
                    emit_group_chunks(g, t, 4, 8, on_act)

            # ---- pipeline body helpers --------------------------------
            p_tiles = {}
            num = {}

            def emit_st(T):
                qb, kt = divmod(T, NKT)
                g, c = divmod(kt, NG)
                st = pst.tile([128, 2 * QB], f32, tag="st", name="st")
                for h in range(2):
                    nc.tensor.matmul(
                        st[:, h * QB:(h + 1) * QB],
                        kT_g[g][h * HD:(h + 1) * HD, c * KT:(c + 1) * KT],
                        qT_g[qb][h * HD:(h + 1) * HD, :],
                        start=True, stop=True,
                        tile_position=(h * HD, 0))
                e_t = ep.tile([128, 2 * QB], P_DT, tag="e", name="e_t")
                nc.scalar.activation(e_t, st, AF.Exp)
                p_t = pp.tile([128, 2 * QB], P_DT, tag="p", name="p_t")
                m0 = msk_s.offset + T * QB
                if T in GP_SET:
                    m = bass.AP(tensor=msk_s.tensor, offset=m0,
                                ap=[list(msk_s.ap[0]), [1, QB]])
                    for h in range(2):
                        nc.gpsimd.tensor_mul(
                            p_t[:, h * QB:(h + 1) * QB],
                            e_t[:, h * QB:(h + 1) * QB], m)
                else:
                    mb = bass.AP(tensor=msk_s.tensor, offset=m0,
                                 ap=[list(msk_s.ap[0]), [0, 2], [1, QB]])
                    e3 = bass.AP(tensor=e_t.tensor, offset=e_t.offset,
                                 ap=[list(e_t.ap[0]), [QB, 2], [1, QB]])
                    p3 = bass.AP(tensor=p_t.tensor, offset=p_t.offset,
                                 ap=[list(p_t.ap[0]), [QB, 2], [1, QB]])
                    nc.vector.tensor_mul(p3, e3, mb)
                p_tiles[T] = p_t

            def emit_av(T):
                qb, kt = divmod(T, NKT)
                if kt == 0:
                    num[qb] = [pacc.tile([HD + 1, QB], f32, tag="num",
                                         name=f"num{h}_{qb}")
                               for h in range(2)]
                p_t = p_tiles.pop(T)
                for h, v_s in ((0, v0_s), (1, v1_s)):
                    nc.tensor.matmul(num[qb][h], v_s[:, kt, :],
                                     p_t[:, h * QB:(h + 1) * QB],
                                     start=(kt == 0), stop=(kt == NKT - 1))

            def stage1(qb):
                """At qb's last AV: drain numerators out of PSUM, start the
                1/den bounce. Returns state for the deferred stages."""
                n0, n1 = num.pop(qb)
                st8 = {}
                for h, n in ((0, n0), (1, n1)):
                    den = small.tile([1, QB], f32, tag=f"den{h}", name="den")
                    nc.vector.tensor_copy(den, n[HD:HD + 1, :])
                    ns = nsb.tile([HD, QB], f32, tag=f"nsb{h}",
                                  name=f"nsb{h}")
                    nc.vector.tensor_copy(ns, n[0:HD, :])
                    rec = small.tile([1, QB], f32, tag=f"rec{h}", name="rec")
                    nc.vector.reciprocal_approx_fast(rec, den)
                    rec_d = drp.tile([1, QB], f32, tag=f"recd{h}",
                                     name="rec_d")
                    nc.sync.dma_start(out=rec_d, in_=rec)
                    bc = bcp.tile([HD, QB], f32, tag=f"bc{h}", name="bc")
                    rb = bass.AP(tensor=rec_d.tensor, offset=rec_d.offset,
                                 ap=[[0, HD], [1, QB]])
                    nc.sync.dma_start(out=bc, in_=rb)
                    st8[h] = (ns, bc)
                return st8

            def make_sh(st8, h, shs):
                def cl():
                    ns, bc = st8[h]
                    sh = shp.tile([HD, QB], bf16, tag=f"sh{h}", name=f"sh{h}")
                    nc.vector.tensor_mul(sh, ns, bc)
                    shs[h] = sh
                return cl

            def make_proj(qb, shs, blk):
                def cl():
                    pr = pacc.tile([128, D], f32, tag="qk", name="pr")
                    nc.tensor.matmul(pr, shs[0][:, blk * 128:(blk + 1) * 128],
                                     wo0_s, start=True, stop=False)
                    nc.tensor.matmul(pr, shs[1][:, blk * 128:(blk + 1) * 128],
                                     wo1_s, start=False, stop=True)
                    rows = slice(qb * QB + blk * 128,
                                 qb * QB + (blk + 1) * 128)
                    o_t = outp.tile([128, D], f32, tag="o", name="o_t")
                    nc.vector.tensor_copy(o_t, pr)
                    nc.gpsimd.dma_start(out=out[rows, :], in_=o_t)
                return cl

            # ---- the flat pipeline ------------------------------------
            emit_group(0, on_act=True)

            # (group, q/k, chunk-range) emission slots: k-chains lead in
            # 4-chunk pieces (kT_g[g] is first needed at tile 4g); the
            # later-needed q-chains go as lighter 2-chunk pieces
            # (qT_g[qb] is first needed at tile 16qb).
            GSLOT = {1: (1, 1, 0, 3), 2: (1, 1, 3, 6), 3: (1, 1, 6, 8),
                     4: (2, 1, 0, 3), 5: (2, 1, 3, 6), 6: (2, 1, 6, 8),
                     7: (3, 1, 0, 3), 8: (3, 1, 3, 6), 9: (3, 1, 6, 8),
                     10: (1, 0, 0, 2), 11: (1, 0, 2, 4),
                     12: (1, 0, 4, 6), 13: (1, 0, 6, 8),
                     14: (2, 0, 0, 2), 15: (2, 0, 2, 4),
                     16: (2, 0, 4, 6), 17: (2, 0, 6, 8),
                     18: (3, 0, 0, 2), 19: (3, 0, 2, 4),
                     20: (3, 0, 4, 6), 21: (3, 0, 6, 8)}

            schedule = defaultdict(list)
            for T in range(NT + LAG):
                if T < NT:
                    if T in GSLOT:
                        emit_group_chunks(*GSLOT[T])
                    if T in (2, 10, 18, 26, 34, 42):
                        issue_mask_batch(16 + (T - 2), 8)
                    emit_st(T)
                for cl in schedule.pop(T, []):
                    cl()
                if T >= LAG:
                    TT = T - LAG
                    emit_av(TT)
                    qb2, kt2 = divmod(TT, NKT)
                    if kt2 == NKT - 1 and qb2 < NQB - 1:
                        st8 = stage1(qb2)
                        shs = {}
                        schedule[T + 2].append(make_sh(st8, 0, shs))
                        schedule[T + 3].append(make_sh(st8, 1, shs))
                        for b in range(4):
                            schedule[T + 5 + 2 * b].append(
                                make_proj(qb2, shs, b))

            # ---- tail: last qb, latency-optimized ---------------------
            # Reciprocals read the den rows straight from PSUM; 1/den is
            # transposed to per-partition layout via a tiny DRAM broadcast
            # bounce (the sync queue is idle here) while the unscaled
            # per-head projections keep the PE busy; the final scaling is
            # split across ACT + DVE.
            qb = NQB - 1
            n0, n1 = num.pop(qb)
            # keep the PE's HAM clock gate open across the tail's gap
            # (idle > ~3.4us re-throttles to 1.2 GHz and doubles the
            # projection matmul times)
            for i in range(3):
                warm_ps = pacc.tile([128, QB], f32, tag="qk",
                                    name=f"twarm{i}")
                nc.tensor.matmul(warm_ps, warm_in[:, 0:128], warm_in,
                                 start=True, stop=True)
            # the 1/den -> DRAM-broadcast-transpose chain is the longest
            # pole; pipeline it per head (ACT drains h0's den row — it's
            # idle after the last exp — while DVE drains h1's), bouncing
            # each half through DRAM as soon as its reciprocal lands.
            dd = small.tile([1, 2 * QB], f32, tag="dd", name="dd")
            nc.scalar.activation(dd[:, 0:QB], n0[HD:HD + 1, :], AF.Identity)
            nc.vector.tensor_copy(dd[:, QB:2 * QB], n1[HD:HD + 1, :])
            # two more keep-warm matmuls chained on dd so they fill the
            # PE idle window between the last AV and the projections
            for i in range(2):
                warm_ps = pacc.tile([128, QB], f32, tag="qk",
                                    name=f"twarm2_{i}")
                nc.tensor.matmul(warm_ps, dd[:, 0:128], dd[:, 0:QB],
                                 start=True, stop=True)
            rec = small.tile([1, 2 * QB], f32, tag="rec2", name="rec")
            rec_d2 = drp.tile([1, 2 * QB], f32, tag="recd2", name="rec_d2")
            rec_t = small.tile([128, 8], f32, tag="rect", name="rec_t")
            for h in range(2):
                sl = slice(h * QB, (h + 1) * QB)
                nc.vector.reciprocal_approx_fast(rec[:, sl], dd[:, sl])
                nc.sync.dma_start(out=rec_d2[:, sl], in_=rec[:, sl])
                nc.sync.dma_start(
                    out=rec_t[:, 4 * h:4 * h + 4],
                    in_=bass.AP(tensor=rec_d2.tensor,
                                offset=rec_d2.offset + h * QB,
                                ap=[[1, 128], [128, 4]]))
            nss = []
            for h, n in ((0, n0), (1, n1)):
                ns = nsb.tile([HD, QB], bf16, tag=f"nst{h}", name=f"nst{h}")
                nc.vector.tensor_copy(ns, n[0:HD, :])
                nss.append(ns)
            prs = []
            for b in (0, 1):
                st_blk = pst.tile([128, 2 * QB], f32, tag="st", name="st_pr")
                prs.append((st_blk[:, 0:QB], st_blk[:, QB:2 * QB]))
            q0 = pacc.tile([128, D], f32, tag="qk", name="prq0")
            q1 = pacc.tile([128, D], f32, tag="qk", name="prq1")
            prs.append((q0, q1))

            def proj_mm(b):
                for h, ns, wo_s in ((0, nss[0], wo0_s), (1, nss[1], wo1_s)):
                    nc.tensor.matmul(prs[b][h],
                                     ns[:, b * 128:(b + 1) * 128],
                                     wo_s, start=True, stop=True)

            def scale_blk(b):
                t0 = small.tile([128, D], f32, tag="t0", name="t0")
                nc.scalar.mul(t0, prs[b][0], rec_t[:, b:b + 1])
                o_t = outp.tile([128, D], f32, tag="o", name="o_t")
                nc.vector.scalar_tensor_tensor(
                    o_t, prs[b][1], rec_t[:, 4 + b:4 + b + 1], t0,
                    mybir.AluOpType.mult, mybir.AluOpType.add)
                rows = slice(qb * QB + b * 128, qb * QB + (b + 1) * 128)
                # tail stores on sync: gpsimd's stream then ends with the
                # in-loop outs, so its ~3us SWDGE drain overlaps the tail
                nc.sync.dma_start(out=out[rows, :], in_=o_t)

            proj_mm(0)
            proj_mm(1)
            proj_mm(2)
            scale_blk(0)
            st_blk = pst.tile([128, 2 * QB], f32, tag="st", name="st_pr")
            prs.append((st_blk[:, 0:QB], st_blk[:, QB:2 * QB]))
            proj_mm(3)
            scale_blk(1)
            scale_blk(2)
            scale_blk(3)

    nc.compile()
    return nc


def _get_compiled():
    global _compiled
    if _compiled is None:
        _compiled = _build()
    return _compiled


def kernel(gene_emb, expr_emb, V, M, fused_W, fused_b, Wq, bq, Wk, bk,
           out_W, out_b):
    gene_emb = np.asarray(gene_emb, dtype=np.float32)
    expr_emb = np.asarray(expr_emb, dtype=np.float32)
    V = np.asarray(V, dtype=np.float32)
    M = np.asarray(M, dtype=np.float32)
    fused_W = np.asarray(fused_W, dtype=np.float32)
    fused_b = np.asarray(fused_b, dtype=np.float32)
    Wq_ = np.asarray(Wq, dtype=np.float32)
    bq_ = np.asarray(bq, dtype=np.float32)
    Wk_ = np.asarray(Wk, dtype=np.float32)
    bk_ = np.asarray(bk, dtype=np.float32)
    out_W = np.asarray(out_W, dtype=np.float32)
    out_b = np.asarray(out_b, dtype=np.float32)

    nc = _get_compiled()

    import ml_dtypes
    m_np = ml_dtypes.bfloat16
    p_np = ml_dtypes.bfloat16
    f8_np = ml_dtypes.float8_e4m3

    def to_f8(a):
        return np.clip(a, -240.0, 240.0).astype(f8_np)

    fW = fused_W.astype(np.float64)
    Wqc = (fW @ Wq_.astype(np.float64)) * SCALE * QSH
    bqc = (fused_b.astype(np.float64) @ Wq_.astype(np.float64) + bq_) * SCALE
    Wkc = (fW @ Wk_.astype(np.float64)) * KSH
    bkc = fused_b.astype(np.float64) @ Wk_.astype(np.float64) + bk_

    def chunk_major(a, nch):  # [nch*128, F] -> [128, nch, F]
        F = a.shape[1]
        return np.ascontiguousarray(
            a.reshape(nch, 128, F).transpose(1, 0, 2))

    xt_b, mt_b = [], []
    for b in range(B):
        XT = np.concatenate([gene_emb[b], expr_emb[b]], axis=-1).T  # [1024,S]
        xg = XT.reshape(NCH, 128, NG, QB).transpose(1, 2, 0, 3)
        xt_b.append(to_f8(np.ascontiguousarray(
            xg.reshape(128, NG * NCH, QB))))
        mtt = M[b].T.reshape(NKT, KT, NQB, QB).transpose(1, 2, 0, 3)
        mt_b.append(np.ascontiguousarray(
            mtt.reshape(KT, NT * QB)).astype(m_np))

    ones_col = np.ones((S, 1), np.float32)
    in_maps = []
    for d in range(N_CORES):
        b, p = d // 4, d % 4
        h0 = 2 * p
        cols = slice(p * 128, (p + 1) * 128)
        vs = []
        for h in (h0, h0 + 1):
            Vh = np.concatenate([V[b, :, h, :], ones_col], axis=1)  # [S,65]
            vs.append(chunk_major(Vh, NKT).astype(p_np))
        in_maps.append({
            "xt": xt_b[b],
            "mt": mt_b[b],
            "wq": to_f8(chunk_major(Wqc[:, cols].astype(np.float32), NCH)),
            "wk": to_f8(chunk_major(Wkc[:, cols].astype(np.float32), NCH)),
            "bq": bqc[cols].astype(np.float32).reshape(128, 1),
            "bk": bkc[cols].astype(np.float32).reshape(128, 1),
            "v0": vs[0],
            "v1": vs[1],
            "wo0": np.ascontiguousarray(
                out_W[h0 * HD:(h0 + 1) * HD, :]).astype(ml_dtypes.bfloat16),
            "wo1": np.ascontiguousarray(
                out_W[(h0 + 1) * HD:(h0 + 2) * HD, :]).astype(
                    ml_dtypes.bfloat16),
        })

    global _last_results
    n_run = int(os.environ.get("KERNEL_CORES", N_CORES))
    if n_run < N_CORES:
        in_maps = in_maps[:1] * N_CORES
    res = run_bass_kernel_spmd(nc, in_maps[:n_run],
                               core_ids=list(range(n_run)))
    if n_run < N_CORES:
        res.results = list(res.results) * (N_CORES // n_run)
    _last_results = res

    final = np.broadcast_to(out_b, (B, S, D)).astype(np.float32).copy()
    for d in range(N_CORES):
        final[d // 4] += res.results[d]["out"]
    return final


# revision 50
# speedup vs baseline: 1.0094x; 1.0094x over previous
"""ExpressionAttentionLayer Trainium2 kernel (v5).

Math (per reference, algebraically folded):
  fused/q/k projections folded on the host into one [1024,128] weight per
  core; A_bar = softmax(qk)*M / L1 == exp(qk)*M / sum_k(exp(qk)*M) (the
  softmax denominator cancels; logits are tiny so no max-subtraction);
  the key-sum denominator rides as a ones-column appended to V.

Device decomposition: core d = batch d//4, head pair (2*(d%4), 2*(d%4)+1).

v5 vs the 124us baseline (validated piecewise against HW traces):
  * x / Wq / Wk stored fp8e4m3 (halves the 4.2MB x stream; PE speed is
    dtype-independent so this is DMA-only). Weights rescaled by powers of
    two (x64/x16) to stay in fp8 normal range; undone exactly on the
    PSUM->SBUF copy (acc * 2^-6|-4 + bias). DoubleRow is NOT used: HW
    measured 1 row/cycle regardless, and it disables weight-load overlap.
  * Masks preloaded into a static [128, 64, 512] SBUF region with batched
    DMAs (8 tiles per DIRECT2D vs 64 singles at 625ns SP config each).
  * Prologue groups' bias-adds run on ACT (idle until the first exp);
    groups 2/3 are emitted split (q/k on consecutive tiles) to smooth PE.
  * Output DMAs go out on the gpsimd SWDGE queue (sync queue carries only
    the input stream).
  * Warm-up trimmed 13->6 matmuls (HAM needs ~3.4us of activity; the qk
    chains provide the rest) so the first qk chain isn't queued behind it.
  * Tail: reciprocals read the denominator rows straight from PSUM (drops
    two [1,512] copies), and 1/den is transposed via a tiny DRAM
    broadcast bounce instead of 8 serial rank-1 PE matmuls (~3.2us).
"""

import os
import sys
from collections import defaultdict

for _p in ("/opt/trn_rl_repo", "/root/.axon_site/_ro/trn_rl_repo"):
    if os.path.isdir(_p) and _p not in sys.path:
        sys.path.insert(0, _p)

import numpy as np

import concourse.bass as bass
import concourse.mybir as mybir
import concourse.tile as tile
from concourse import bacc
from concourse.bass_utils import run_bass_kernel_spmd

B, S, D, H, HD = 2, 2048, 512, 8, 64
KX = 2 * D
NCH = KX // 128
NG = 4
N_CORES = 8
QB = 512
NQB = S // QB
KT = 128
NKT = S // KT
NT = NQB * NKT
LAG = 4
SCALE = 1.0 / np.sqrt(HD)
QSH, KSH = 64.0, 16.0
NWARM = int(os.environ.get("KERNEL_NWARM", "3"))

f32 = mybir.dt.float32
bf16 = mybir.dt.bfloat16
f8 = mybir.dt.float8e4

M_DT = bf16
P_DT = bf16
X_DT = f8

GP_MOD = int(os.environ.get("KERNEL_GP_MOD", "0"))
GP_OFF = int(os.environ.get("KERNEL_GP_OFF", "3"))
GP_SET = set(T for T in range(NT) if GP_MOD and T % GP_MOD == GP_OFF)

_compiled = None
_last_results = None


def _build():
    nc = bacc.Bacc("TRN2", target_bir_lowering=False, debug=False,
                   num_devices=N_CORES)
    AF = mybir.ActivationFunctionType

    xt = nc.dram_tensor("xt", [128, NG * NCH, QB], X_DT,
                        kind="ExternalInput").ap()
    mt = nc.dram_tensor("mt", [128, NT * QB], M_DT, kind="ExternalInput").ap()
    wq = nc.dram_tensor("wq", [128, NCH, 128], X_DT, kind="ExternalInput").ap()
    wk = nc.dram_tensor("wk", [128, NCH, 128], X_DT, kind="ExternalInput").ap()
    bq = nc.dram_tensor("bq", [128, 1], f32, kind="ExternalInput").ap()
    bk = nc.dram_tensor("bk", [128, 1], f32, kind="ExternalInput").ap()
    v0 = nc.dram_tensor("v0", [128, NKT, HD + 1], P_DT, kind="ExternalInput").ap()
    v1 = nc.dram_tensor("v1", [128, NKT, HD + 1], P_DT, kind="ExternalInput").ap()
    wo0 = nc.dram_tensor("wo0", [HD, D], bf16, kind="ExternalInput").ap()
    wo1 = nc.dram_tensor("wo1", [HD, D], bf16, kind="ExternalInput").ap()
    out = nc.dram_tensor("out", [S, D], f32, kind="ExternalOutput").ap()

    with tile.TileContext(nc) as tc:
        with tc.tile_pool(name="const", bufs=1) as const, \
             tc.tile_pool(name="ep", bufs=8) as ep, \
             tc.tile_pool(name="pp", bufs=8) as pp, \
             tc.tile_pool(name="nsb", bufs=2) as nsb, \
             tc.tile_pool(name="small", bufs=2) as small, \
             tc.tile_pool(name="bcp", bufs=2) as bcp, \
             tc.tile_pool(name="shp", bufs=2) as shp, \
             tc.tile_pool(name="outp", bufs=4) as outp, \
             tc.tile_pool(name="pst", bufs=2, space="PSUM") as pst, \
             tc.tile_pool(name="pacc", bufs=2, space="PSUM") as pacc, \
             tc.tile_pool(name="drp", bufs=2, space="DRAM") as drp:

            # ---- PE warm-up burst + Exp table preload -----------------
            warm_in = const.tile([128, QB], bf16)
            nc.vector.memset(warm_in, 1.0)
            warm_o = const.tile([1, 8], f32)
            for i in range(NWARM):
                warm_ps = pacc.tile([128, QB], f32, tag="qk",
                                    name=f"warm{i}")
                nc.tensor.matmul(warm_ps, warm_in[:, 0:128], warm_in,
                                 start=True, stop=True)
            nc.scalar.activation(warm_o, warm_in[0:1, 0:8], AF.Exp)

            # ---- static input regions ---------------------------------
            xt_s = const.tile([128, NG * NCH, QB], X_DT)
            msk_s = const.tile([128, NT, QB], M_DT)
            wq_s = const.tile([128, NCH, 128], X_DT)
            wk_s = const.tile([128, NCH, 128], X_DT)
            bq_s = const.tile([128, 1], f32)
            bk_s = const.tile([128, 1], f32)
            v0_s = const.tile([128, NKT, HD + 1], P_DT)
            v1_s = const.tile([128, NKT, HD + 1], P_DT)
            wo0_s = const.tile([HD, D], bf16)
            wo1_s = const.tile([HD, D], bf16)

            def issue_xt(g):
                nc.sync.dma_start(out=xt_s[:, g * NCH:(g + 1) * NCH, :],
                                  in_=xt[:, g * NCH:(g + 1) * NCH, :])

            def issue_mask_batch(start, count):
                nc.sync.dma_start(
                    out=msk_s[:, start:start + count, :],
                    in_=mt[:, start * QB:(start + count) * QB])

            # SWDGE warm-up: pay the software queue's startup latency now.
            swdge_warm = drp.tile([1, 8], f32, tag="swarm", name="swarm")
            nc.gpsimd.dma_start(out=swdge_warm, in_=warm_o)

            # sync-queue order tuned so early-needed data isn't stuck
            # behind bulk transfers (the HWDGE queue is FIFO). Weights
            # first (small), then x chunks 0-1 so the first projection
            # chunks can start ~2.5us before the full x group lands.
            nc.sync.dma_start(out=wq_s, in_=wq)
            nc.sync.dma_start(out=wk_s, in_=wk)
            nc.sync.dma_start(out=xt_s[:, 0:2, :], in_=xt[:, 0:2, :])
            nc.sync.dma_start(out=xt_s[:, 2:NCH, :], in_=xt[:, 2:NCH, :])
            nc.sync.dma_start(out=bq_s, in_=bq)
            nc.sync.dma_start(out=bk_s, in_=bk)
            issue_mask_batch(0, 8)
            issue_xt(1)
            nc.sync.dma_start(out=v0_s, in_=v0)
            nc.sync.dma_start(out=v1_s, in_=v1)
            issue_xt(2)
            issue_mask_batch(8, 4)
            issue_xt(3)
            issue_mask_batch(12, 4)
            nc.sync.dma_start(out=wo0_s, in_=wo0)
            nc.sync.dma_start(out=wo1_s, in_=wo1)

            qT_g = [const.tile([128, QB], bf16, name=f"qT{g}")
                    for g in range(NG)]
            kT_g = [const.tile([128, QB], bf16, name=f"kT{g}")
                    for g in range(NG)]

            group_accs = {}

            def emit_group_chunks(g, t, lo, hi, on_act=False):
                """Chunks [lo,hi) of one fp8 projection chain (t=0: q,
                t=1: k), emitted piecewise so the Tile scheduler can't
                hoist a whole 1.7us chain ahead of a ready ST and starve
                ACT. The power-of-2 weight rescale is undone on the
                PSUM->SBUF copy (ACT for prologue pieces, DVE in-loop)."""
                w_s, b_s, sh, dst = (
                    (wq_s, bq_s, 1.0 / QSH, qT_g[g]) if t == 0
                    else (wk_s, bk_s, 1.0 / KSH, kT_g[g]))
                if lo == 0:
                    group_accs[(g, t)] = pacc.tile([128, QB], f32, tag="qk",
                                                   name=f"qk{t}_{g}")
                acc = group_accs[(g, t)]
                for i in range(lo, hi):
                    nc.tensor.matmul(
                        acc, w_s[:, i, :], xt_s[:, g * NCH + i, :],
                        start=(i == 0), stop=(i == NCH - 1))
                if hi == NCH:
                    if on_act:
                        if t == 1:
                            # ST(0) only needs kT cols 0-127; unblock it
                            # before the rest of the bias copy
                            nc.scalar.activation(dst[:, 0:KT],
                                                 acc[:, 0:KT],
                                                 AF.Identity,
                                                 bias=b_s, scale=sh)
                            nc.scalar.activation(dst[:, KT:QB],
                                                 acc[:, KT:QB],
                                                 AF.Identity,
                                                 bias=b_s, scale=sh)
                        else:
                            nc.scalar.activation(dst, acc, AF.Identity,
                                                 bias=b_s, scale=sh)
                    else:
                        nc.vector.tensor_scalar(dst, acc, sh, b_s,
                                                mybir.AluOpType.mult,
                                                mybir.AluOpType.add)

            def emit_group(g, on_act=False):
                for t in (0, 1):
                    emit_group_chunks(g, t, 0, 4, on_act)
                    emit_group_chunks(g, t, 4, 8, on_act)

            # ---- pipeline body helpers --------------------------------
            p_tiles = {}
            num = {}

            def emit_st(T):
                qb, kt = divmod(T, NKT)
                g, c = divmod(kt, NG)
                st = pst.tile([128, 2 * QB], f32, tag="st", name="st")
                for h in range(2):
                    nc.tensor.matmul(
                        st[:, h * QB:(h + 1) * QB],
                        kT_g[g][h * HD:(h + 1) * HD, c * KT:(c + 1) * KT],
                        qT_g[qb][h * HD:(h + 1) * HD, :],
                        start=True, stop=True,
                        tile_position=(h * HD, 0))
                e_t = ep.tile([128, 2 * QB], P_DT, tag="e", name="e_t")
                nc.scalar.activation(e_t, st, AF.Exp)
                p_t = pp.tile([128, 2 * QB], P_DT, tag="p", name="p_t")
                m0 = msk_s.offset + T * QB
                if T in GP_SET:
                    m = bass.AP(tensor=msk_s.tensor, offset=m0,
                                ap=[list(msk_s.ap[0]), [1, QB]])
                    for h in range(2):
                        nc.gpsimd.tensor_mul(
                            p_t[:, h * QB:(h + 1) * QB],
                            e_t[:, h * QB:(h + 1) * QB], m)
                else:
                    mb = bass.AP(tensor=msk_s.tensor, offset=m0,
                                 ap=[list(msk_s.ap[0]), [0, 2], [1, QB]])
                    e3 = bass.AP(tensor=e_t.tensor, offset=e_t.offset,
                                 ap=[list(e_t.ap[0]), [QB, 2], [1, QB]])
                    p3 = bass.AP(tensor=p_t.tensor, offset=p_t.offset,
                                 ap=[list(p_t.ap[0]), [QB, 2], [1, QB]])
                    nc.vector.tensor_mul(p3, e3, mb)
                p_tiles[T] = p_t

            def emit_av(T):
                qb, kt = divmod(T, NKT)
                if kt == 0:
                    num[qb] = [pacc.tile([HD + 1, QB], f32, tag="num",
                                         name=f"num{h}_{qb}")
                               for h in range(2)]
                p_t = p_tiles.pop(T)
                for h, v_s in ((0, v0_s), (1, v1_s)):
                    nc.tensor.matmul(num[qb][h], v_s[:, kt, :],
                                     p_t[:, h * QB:(h + 1) * QB],
                                     start=(kt == 0), stop=(kt == NKT - 1))

            def stage1(qb):
                """At qb's last AV: drain numerators out of PSUM, start the
                1/den bounce. Returns state for the deferred stages."""
                n0, n1 = num.pop(qb)
                st8 = {}
                for h, n in ((0, n0), (1, n1)):
                    den = small.tile([1, QB], f32, tag=f"den{h}", name="den")
                    nc.vector.tensor_copy(den, n[HD:HD + 1, :])
                    ns = nsb.tile([HD, QB], f32, tag=f"nsb{h}",
                                  name=f"nsb{h}")
                    nc.vector.tensor_copy(ns, n[0:HD, :])
                    rec = small.tile([1, QB], f32, tag=f"rec{h}", name="rec")
                    nc.vector.reciprocal_approx_fast(rec, den)
                    rec_d = drp.tile([1, QB], f32, tag=f"recd{h}",
                                     name="rec_d")
                    nc.sync.dma_start(out=rec_d, in_=rec)
                    bc = bcp.tile([HD, QB], f32, tag=f"bc{h}", name="bc")
                    rb = bass.AP(tensor=rec_d.tensor, offset=rec_d.offset,
                                 ap=[[0, HD], [1, QB]])
                    nc.sync.dma_start(out=bc, in_=rb)
                    st8[h] = (ns, bc)
                return st8

            def make_sh(st8, h, shs):
                def cl():
                    ns, bc = st8[h]
                    sh = shp.tile([HD, QB], bf16, tag=f"sh{h}", name=f"sh{h}")
                    nc.vector.tensor_mul(sh, ns, bc)
                    shs[h] = sh
                return cl

            def make_proj(qb, shs, blk):
                def cl():
                    pr = pacc.tile([128, D], f32, tag="qk", name="pr")
                    nc.tensor.matmul(pr, shs[0][:, blk * 128:(blk + 1) * 128],
                                     wo0_s, start=True, stop=False)
                    nc.tensor.matmul(pr, shs[1][:, blk * 128:(blk + 1) * 128],
                                     wo1_s, start=False, stop=True)
                    rows = slice(qb * QB + blk * 128,
                                 qb * QB + (blk + 1) * 128)
                    o_t = outp.tile([128, D], f32, tag="o", name="o_t")
                    nc.vector.tensor_copy(o_t, pr)
                    nc.gpsimd.dma_start(out=out[rows, :], in_=o_t)
                return cl

            # ---- the flat pipeline ------------------------------------
            emit_group(0, on_act=True)

            # (group, q/k, chunk-range) emission slots: k-chains lead in
            # 4-chunk pieces (kT_g[g] is first needed at tile 4g); the
            # later-needed q-chains go as lighter 2-chunk pieces
            # (qT_g[qb] is first needed at tile 16qb).
            GSLOT = {1: (1, 1, 0, 3), 2: (1, 1, 3, 6), 3: (1, 1, 6, 8),
                     4: (2, 1, 0, 3), 5: (2, 1, 3, 6), 6: (2, 1, 6, 8),
                     7: (3, 1, 0, 3), 8: (3, 1, 3, 6), 9: (3, 1, 6, 8),
                     10: (1, 0, 0, 2), 11: (1, 0, 2, 4),
                     12: (1, 0, 4, 6), 13: (1, 0, 6, 8),
                     14: (2, 0, 0, 2), 15: (2, 0, 2, 4),
                     16: (2, 0, 4, 6), 17: (2, 0, 6, 8),
                     18: (3, 0, 0, 2), 19: (3, 0, 2, 4),
                     20: (3, 0, 4, 6), 21: (3, 0, 6, 8)}

            schedule = defaultdict(list)
            for T in range(NT + LAG):
                if T < NT:
                    if T in GSLOT:
                        emit_group_chunks(*GSLOT[T])
                    if T in (2, 10, 18, 26, 34, 42):
                        issue_mask_batch(16 + (T - 2), 8)
                    emit_st(T)
                for cl in schedule.pop(T, []):
                    cl()
                if T >= LAG:
                    TT = T - LAG
                    emit_av(TT)
                    qb2, kt2 = divmod(TT, NKT)
                    if kt2 == NKT - 1 and qb2 < NQB - 1:
                        st8 = stage1(qb2)
                        shs = {}
                        schedule[T + 2].append(make_sh(st8, 0, shs))
                        schedule[T + 3].append(make_sh(st8, 1, shs))
                        for b in range(4):
                            schedule[T + 5 + 2 * b].append(
                                make_proj(qb2, shs, b))

            # ---- tail: last qb, latency-optimized ---------------------
            # Reciprocals read the den rows straight from PSUM; 1/den is
            # transposed to per-partition layout via a tiny DRAM broadcast
            # bounce (the sync queue is idle here) while the unscaled
            # per-head projections keep the PE busy; the final scaling is
            # split across ACT + DVE.
            qb = NQB - 1
            n0, n1 = num.pop(qb)
            # keep the PE's HAM clock gate open across the tail's gap
            # (idle > ~3.4us re-throttles to 1.2 GHz and doubles the
            # projection matmul times)
            for i in range(3):
                warm_ps = pacc.tile([128, QB], f32, tag="qk",
                                    name=f"twarm{i}")
                nc.tensor.matmul(warm_ps, warm_in[:, 0:128], warm_in,
                                 start=True, stop=True)
            # the 1/den -> DRAM-broadcast-transpose chain is the longest
            # pole; pipeline it per head (ACT drains h0's den row — it's
            # idle after the last exp — while DVE drains h1's), bouncing
            # each half through DRAM as soon as its reciprocal lands.
            dd = small.tile([1, 2 * QB], f32, tag="dd", name="dd")
            nc.scalar.activation(dd[:, 0:QB], n0[HD:HD + 1, :], AF.Identity)
            nc.vector.tensor_copy(dd[:, QB:2 * QB], n1[HD:HD + 1, :])
            # two more keep-warm matmuls chained on dd so they fill the
            # PE idle window between the last AV and the projections
            for i in range(2):
                warm_ps = pacc.tile([128, QB], f32, tag="qk",
                                    name=f"twarm2_{i}")
                nc.tensor.matmul(warm_ps, dd[:, 0:128], dd[:, 0:QB],
                                 start=True, stop=True)
            rec = small.tile([1, 2 * QB], f32, tag="rec2", name="rec")
            rec_d2 = drp.tile([1, 2 * QB], f32, tag="recd2", name="rec_d2")
            rec_t = small.tile([128, 8], f32, tag="rect", name="rec_t")
            for h in range(2):
                sl = slice(h * QB, (h + 1) * QB)
                nc.vector.reciprocal_approx_fast(rec[:, sl], dd[:, sl])
                nc.sync.dma_start(out=rec_d2[:, sl], in_=rec[:, sl])
                nc.sync.dma_start(
                    out=rec_t[:, 4 * h:4 * h + 4],
                    in_=bass.AP(tensor=rec_d2.tensor,
                                offset=rec_d2.offset + h * QB,
                                ap=[[1, 128], [128, 4]]))
            nss = []
            for h, n in ((0, n0), (1, n1)):
                ns = nsb.tile([HD, QB], bf16, tag=f"nst{h}", name=f"nst{h}")
                nc.vector.tensor_copy(ns, n[0:HD, :])
                nss.append(ns)
            prs = []
            for b in (0, 1):
                st_blk = pst.tile([128, 2 * QB], f32, tag="st", name="st_pr")
                prs.append((st_blk[:, 0:QB], st_blk[:, QB:2 * QB]))
            q0 = pacc.tile([128, D], f32, tag="qk", name="prq0")
            q1 = pacc.tile([128, D], f32, tag="qk", name="prq1")
            prs.append((q0, q1))

            def proj_mm(b):
                for h, ns, wo_s in ((0, nss[0], wo0_s), (1, nss[1], wo1_s)):
                    nc.tensor.matmul(prs[b][h],
                                     ns[:, b * 128:(b + 1) * 128],
                                     wo_s, start=True, stop=True)

            def scale_blk(b):
                t0 = small.tile([128, D], f32, tag="t0", name="t0")
                nc.scalar.mul(t0, prs[b][0], rec_t[:, b:b + 1])
                o_t = outp.tile([128, D], f32, tag="o", name="o_t")
                nc.vector.scalar_tensor_tensor(
                    o_t, prs[b][1], rec_t[:, 4 + b:4 + b + 1], t0,
                    mybir.AluOpType.mult, mybir.AluOpType.add)
                rows = slice(qb * QB + b * 128, qb * QB + (b + 1) * 128)
                # tail stores on sync: gpsimd's stream then ends with the
                # in-loop outs, so its ~3us SWDGE drain overlaps the tail
                nc.sync.dma_start(out=out[rows, :], in_=o_t)

            proj_mm(0)
            proj_mm(1)
            proj_mm(2)
            scale_blk(0)
            st_blk = pst.tile([128, 2 * QB], f32, tag="st", name="st_pr")
            prs.append((st_blk[:, 0:QB], st_blk[:, QB:2 * QB]))
            proj_mm(3)
            scale_blk(1)
            scale_blk(2)
            scale_blk(3)

    nc.compile()
    return nc


def _get_compiled():
    global _compiled
    if _compiled is None:
        _compiled = _build()
    return _compiled


def kernel(gene_emb, expr_emb, V, M, fused_W, fused_b, Wq, bq, Wk, bk,
           out_W, out_b):
    gene_emb = np.asarray(gene_emb, dtype=np.float32)
    expr_emb = np.asarray(expr_emb, dtype=np.float32)
    V = np.asarray(V, dtype=np.float32)
    M = np.asarray(M, dtype=np.float32)
    fused_W = np.asarray(fused_W, dtype=np.float32)
    fused_b = np.asarray(fused_b, dtype=np.float32)
    Wq_ = np.asarray(Wq, dtype=np.float32)
    bq_ = np.asarray(bq, dtype=np.float32)
    Wk_ = np.asarray(Wk, dtype=np.float32)
    bk_ = np.asarray(bk, dtype=np.float32)
    out_W = np.asarray(out_W, dtype=np.float32)
    out_b = np.asarray(out_b, dtype=np.float32)

    nc = _get_compiled()

    import ml_dtypes
    m_np = ml_dtypes.bfloat16
    p_np = ml_dtypes.bfloat16
    f8_np = ml_dtypes.float8_e4m3

    def to_f8(a):
        return np.clip(a, -240.0, 240.0).astype(f8_np)

    fW = fused_W.astype(np.float64)
    Wqc = (fW @ Wq_.astype(np.float64)) * SCALE * QSH
    bqc = (fused_b.astype(np.float64) @ Wq_.astype(np.float64) + bq_) * SCALE
    Wkc = (fW @ Wk_.astype(np.float64)) * KSH
    bkc = fused_b.astype(np.float64) @ Wk_.astype(np.float64) + bk_

    def chunk_major(a, nch):  # [nch*128, F] -> [128, nch, F]
        F = a.shape[1]
        return np.ascontiguousarray(
            a.reshape(nch, 128, F).transpose(1, 0, 2))

    xt_b, mt_b = [], []
    for b in range(B):
        XT = np.concatenate([gene_emb[b], expr_emb[b]], axis=-1).T  # [1024,S]
        xg = XT.reshape(NCH, 128, NG, QB).transpose(1, 2, 0, 3)
        xt_b.append(to_f8(np.ascontiguousarray(
            xg.reshape(128, NG * NCH, QB))))
        mtt = M[b].T.reshape(NKT, KT, NQB, QB).transpose(1, 2, 0, 3)
        mt_b.append(np.ascontiguousarray(
            mtt.reshape(KT, NT * QB)).astype(m_np))

    ones_col = np.ones((S, 1), np.float32)
    in_maps = []
    for d in range(N_CORES):
        b, p = d // 4, d % 4
        h0 = 2 * p
        cols = slice(p * 128, (p + 1) * 128)
        vs = []
        for h in (h0, h0 + 1):
            Vh = np.concatenate([V[b, :, h, :], ones_col], axis=1)  # [S,65]
            vs.append(chunk_major(Vh, NKT).astype(p_np))
        in_maps.append({
            "xt": xt_b[b],
            "mt": mt_b[b],
            "wq": to_f8(chunk_major(Wqc[:, cols].astype(np.float32), NCH)),
            "wk": to_f8(chunk_major(Wkc[:, cols].astype(np.float32), NCH)),
            "bq": bqc[cols].astype(np.float32).reshape(128, 1),
            "bk": bkc[cols].astype(np.float32).reshape(128, 1),
            "v0": vs[0],
            "v1": vs[1],
            "wo0": np.ascontiguousarray(
                out_W[h0 * HD:(h0 + 1) * HD, :]).astype(ml_dtypes.bfloat16),
            "wo1": np.ascontiguousarray(
                out_W[(h0 + 1) * HD:(h0 + 2) * HD, :]).astype(
                    ml_dtypes.bfloat16),
        })

    global _last_results
    n_run = int(os.environ.get("KERNEL_CORES", N_CORES))
    if n_run < N_CORES:
        in_maps = in_maps[:1] * N_CORES
    res = run_bass_kernel_spmd(nc, in_maps[:n_run],
                               core_ids=list(range(n_run)))
    if n_run < N_CORES:
        res.results = list(res.results) * (N_CORES // n_run)
    _last_results = res

    final = np.broadcast_to(out_b, (B, S, D)).astype(np.float32).copy()
    for d in range(N_CORES):
        final[d // 4] += res.results[d]["out"]
    return final
